# revision 10
# baseline (speedup 1.0000x reference)
"""Multi-head self-attention (RoPE, causal) on 8 Trainium2 NeuronCores.

Sharding: tensor-parallel over heads. Each core owns 2 of 16 heads:
  - QKV projections column-sharded (each core computes its 128 features)
  - V^T computed directly (x-block stationary) so no PE transposes
  - attention per (batch, head) pair on-core, scores kept transposed
    [tk, tq]; the two heads' K=64 score matmuls run concurrently in the
    PE array (row tiling via base_partition 0/64) into one [128,2,512]
    PSUM tile, so softmax exp is a single ACT op per j-block
  - softmax denominator via a ones-column appended to V^T
  - per-batch AllToAll switches from head- to token-sharding; the output
    projection for batch b overlaps attention of batch b+1
  - projection/output-projection work is sliced into ~8-matmul quanta and
    one quantum is emitted per attention j-iteration, so the PE always
    has ready work while ACT walks the exp stream (keeps HAM at 2.4 GHz)

dtypes: bf16 operands everywhere (FWL stays enabled), fp32 PSUM accum.
"""

from collections import deque

import numpy as np
import ml_dtypes

import concourse.bacc as bacc
import concourse.mybir as mybir
import concourse.tile as tile
from concourse import bass_utils

F32 = mybir.dt.float32
BF16 = mybir.dt.bfloat16

B, T, D = 4, 2048, 1024
H, DH = 16, 64
N_CORES = 8
HPC = H // N_CORES            # heads per core = 2
EC = HPC * DH                 # feature slice per core = 128
NT = B * T                    # 8192 tokens
THETA = 10000.0
NB = T // 128                 # 16 tk blocks per batch
TPB = T // N_CORES            # 256 tokens per (core, batch) after A2A

_CACHE = {}
last_results = None


def _build_program():
    nc = bacc.Bacc("TRN2", debug=False, target_bir_lowering=False,
                   num_devices=N_CORES)

    xt_d = nc.dram_tensor("xt", [128, 8, NT], BF16, kind="ExternalInput")
    wq_d = nc.dram_tensor("wq", [128, 8, EC], BF16, kind="ExternalInput")
    wk_d = nc.dram_tensor("wk", [128, 8, EC], BF16, kind="ExternalInput")
    wv_d = nc.dram_tensor("wv", [128, 8, EC], BF16, kind="ExternalInput")
    wo_d = nc.dram_tensor("wo", [128, 8, D], BF16, kind="ExternalInput")
    cos_d = nc.dram_tensor("cosb", [128, T], F32, kind="ExternalInput")
    sin_d = nc.dram_tensor("sinb", [128, T], F32, kind="ExternalInput")
    rotm_d = nc.dram_tensor("rotm", [128, 128], BF16, kind="ExternalInput")
    tri_d = nc.dram_tensor("trimask", [128, 128], BF16, kind="ExternalInput")
    y_d = nc.dram_tensor("y", [B * TPB, D], F32, kind="ExternalOutput")

    with tile.TileContext(nc) as tc:
        with (
            tc.tile_pool(name="consts", bufs=1) as consts,
            tc.tile_pool(name="big", bufs=1) as big,
            tc.tile_pool(name="xp", bufs=2) as xp,
            tc.tile_pool(name="stage", bufs=2) as stage,
            tc.tile_pool(name="expp", bufs=4) as expp,
            tc.tile_pool(name="outp", bufs=3) as outp,
            tc.tile_pool(name="oall", bufs=2) as oallp,
            tc.tile_pool(name="pp", bufs=3, space="PSUM") as pp,
            tc.tile_pool(name="pv", bufs=2, space="PSUM") as pvp,
            tc.tile_pool(name="dram", bufs=1, space="DRAM") as dram,
        ):
            # ---- constants ----
            cos_sb = consts.tile([128, T], F32)
            sin_sb = consts.tile([128, T], F32)
            rotm_sb = consts.tile([128, 128], BF16)
            tri2_sb = consts.tile([128, 2, 128], BF16)
            nc.sync.dma_start(cos_sb[:], cos_d[:, :])
            nc.sync.dma_start(sin_sb[:], sin_d[:, :])
            nc.sync.dma_start(rotm_sb[:], rotm_d[:, :])
            nc.sync.dma_start(tri2_sb[:, 0, :], tri_d[:, :])
            nc.sync.dma_start(tri2_sb[:, 1, :], tri_d[:, :])

            wq_sb = consts.tile([128, 8, EC], BF16)
            wk_sb = consts.tile([128, 8, EC], BF16)
            wv_sb = consts.tile([128, 8, EC], BF16)
            wo_sb = consts.tile([128, 8, D], BF16)
            nc.sync.dma_start(wq_sb[:], wq_d[:, :, :])
            nc.sync.dma_start(wk_sb[:], wk_d[:, :, :])
            nc.sync.dma_start(wv_sb[:], wv_d[:, :, :])
            nc.sync.dma_start(wo_sb[:], wo_d[:, :, :])

            # ---- persistent tensors ----
            qT = big.tile([128, NT], BF16, tag="qT")
            kT = big.tile([128, NT], BF16, tag="kT")
            # V^T per (pair, tk-block): [tk=128, 128] cols 0-63 = v,
            # col 64 = ones (softmax denominator), cols 65-127 unused
            # (full-128 stationary keeps FWL enabled on the PV matmuls).
            vext = big.tile([128, HPC * B, NB, 128], BF16, tag="vext")
            nc.vector.memset(vext[:, :, :, 64], 1.0)

            a2a_in = [dram.tile([N_CORES, 128, TPB], BF16, tag=f"a2ai{b}",
                                name=f"a2ai{b}") for b in range(B)]
            a2a_out = [dram.tile([N_CORES, 128, TPB], BF16, tag=f"a2ao{b}",
                                 name=f"a2ao{b}") for b in range(B)]
            rg = [list(range(N_CORES))]

            # ============ QKV projections + RoPE for one 512-token chunk,
            # sliced into small PE quanta so they weave between attention
            # j-iterations without starving the ACT exp stream.
            def proj_chunk_steps(ci):
                t0 = 512 * ci
                bb = t0 // T
                state = {}

                def load_x():
                    xt = xp.tile([128, 8, 512], BF16, tag="x",
                                 name=f"x_{ci}")
                    nc.sync.dma_start(xt[:], xt_d[:, :, t0:t0 + 512])
                    state["xt"] = xt

                def _proj(w_sb, nm):
                    pt = pp.tile([128, 1024], F32, tag="pp",
                                 name=f"p{nm}_{ci}")[:, 0:512]
                    for ko in range(8):
                        nc.tensor.matmul(pt, w_sb[:, ko, :],
                                         state["xt"][:, ko, :],
                                         start=(ko == 0), stop=(ko == 7))
                    state["p" + nm] = pt

                def _rope(nm):
                    raw = stage.tile([128, 512], BF16, tag="raw" + nm,
                                     name=f"raw{nm}_{ci}")
                    nc.vector.tensor_copy(raw[:], state["p" + nm])
                    rot = pp.tile([128, 1024], F32, tag="pp",
                                  name=f"r{nm}_{ci}")[:, 0:512]
                    nc.tensor.matmul(rot, rotm_sb[:], raw[:],
                                     start=True, stop=True)
                    state["raw" + nm] = raw
                    state["rot" + nm] = rot

                def _combine(nm, dest):
                    s0 = t0 % T
                    raw, rot = state["raw" + nm], state["rot" + nm]
                    t1 = stage.tile([128, 512], F32, tag="t1" + nm,
                                    name=f"t1{nm}_{ci}")
                    nc.gpsimd.tensor_tensor(
                        t1[:], raw[:], cos_sb[:, s0:s0 + 512],
                        mybir.AluOpType.mult)
                    t2 = stage.tile([128, 512], F32, tag="t2" + nm,
                                    name=f"t2{nm}_{ci}")
                    nc.vector.tensor_tensor(
                        t2[:], rot[:], sin_sb[:, s0:s0 + 512],
                        mybir.AluOpType.mult)
                    nc.gpsimd.tensor_tensor(
                        dest[:, t0:t0 + 512], t1[:], t2[:],
                        mybir.AluOpType.add)

                def _vt(tb):
                    if tb == 0:
                        state["vt"] = pp.tile([128, 1024], F32, tag="pp",
                                              name=f"vt_{ci}")[:, 0:512]
                    pvt_ = state["vt"]
                    for ko in range(8):
                        nc.tensor.matmul(
                            pvt_[:, 128 * tb:128 * tb + 128],
                            state["xt"][:, ko, 128 * tb:128 * tb + 128],
                            wv_sb[:, ko, :],
                            start=(ko == 0), stop=(ko == 7))

                def _vcopy(tb):
                    jg = (t0 % T) // 128 + tb
                    src = state["vt"][:, 128 * tb:128 * tb + 128]
                    nc.vector.tensor_copy(
                        vext[:, bb * HPC:bb * HPC + 2, jg, 0:64],
                        src.rearrange("p (h w) -> p h w", h=2))

                return [
                    load_x,
                    lambda: _proj(wq_sb, "q"),
                    lambda: _rope("q"),
                    lambda: (_combine("q", qT), _proj(wk_sb, "k"))[-1],
                    lambda: _rope("k"),
                    lambda: _combine("k", kT),
                    lambda: _vt(0),
                    lambda: _vt(1),
                    lambda: (_vcopy(0), _vt(2))[-1],
                    lambda: (_vcopy(1), _vt(3))[-1],
                    lambda: (_vcopy(2), _vcopy(3))[-1],
                ]

            # ============ attention for one (batch, 512-wide tq chunk)
            def attn_chunk(bb, c, steps):
                boff = bb * T
                q0 = boff + 512 * c
                jmax = 4 * c + 4
                pvt = [pvp.tile([128, 512], F32, tag="pv",
                                name=f"pv_{bb}_{c}_{h}")
                       for h in range(HPC)]
                for j in range(jmax):
                    lo = max(0, 128 * (j - 4 * c))
                    sc = pp.tile([128, 2, 512], F32, tag="pp",
                                 name=f"s_{bb}_{c}_{j}")
                    for h in range(HPC):
                        nc.tensor.matmul(
                            sc[:, h, lo:512],
                            kT[64 * h:64 * h + 64,
                               boff + 128 * j:boff + 128 * j + 128],
                            qT[64 * h:64 * h + 64, q0 + lo:q0 + 512],
                            start=True, stop=True)
                    ex = expp.tile([128, 2, 512], BF16, tag="e",
                                   name=f"e_{bb}_{c}_{j}")
                    nc.scalar.activation(
                        ex[:, :, lo:512], sc[:, :, lo:512],
                        mybir.ActivationFunctionType.Exp, scale=0.125)
                    if j >= 4 * c:  # diagonal block: causal mask
                        nc.vector.tensor_tensor(
                            ex[:, :, lo:lo + 128], ex[:, :, lo:lo + 128],
                            tri2_sb[:], mybir.AluOpType.mult)
                    for h in range(HPC):
                        nc.tensor.matmul(
                            pvt[h][:, lo:512],
                            vext[:, bb * HPC + h, j, :],
                            ex[:, h, lo:512],
                            start=(j == 0), stop=(j == jmax - 1))
                    if steps:
                        steps.popleft()()
                # normalize + stage for A2A
                for h in range(HPC):
                    unn = outp.tile([64, 512], BF16, tag="unn")
                    nc.vector.tensor_copy(unn[:], pvt[h][0:64, :])
                    dn0 = stage.tile([1, 512], F32, tag="dn0",
                                     name=f"dn0_{bb}_{c}_{h}")
                    nc.vector.tensor_copy(dn0[:], pvt[h][64:65, :])
                    rec = stage.tile([1, 512], F32, tag="rec",
                                     name=f"rec_{bb}_{c}_{h}")
                    nc.vector.reciprocal_approx_fast(rec[:], dn0[:])
                    recb = outp.tile([64, 512], F32, tag="recb")
                    nc.gpsimd.partition_broadcast(recb[:], rec[:])
                    ao = outp.tile([64, 512], BF16, tag="ao")
                    nc.vector.tensor_tensor(ao[:], unn[:], recb[:],
                                            mybir.AluOpType.mult)
                    for half in range(2):
                        nc.sync.dma_start(
                            a2a_in[bb][2 * c + half,
                                       64 * h:64 * h + 64, :],
                            ao[:, 256 * half:256 * half + 256])
                for _ in range(2):
                    if steps:
                        steps.popleft()()

            # ============ output projection for one batch (post-A2A)
            def oproj_steps(bb):
                state = [None]

                def first():
                    oall_b = oallp.tile([128, 8, TPB], BF16, tag="oall",
                                        name=f"oall_{bb}")
                    nc.sync.dma_start(
                        oall_b[:], a2a_out[bb][:].rearrange("s p t -> p s t"))
                    state[0] = oall_b

                def one(eo, tb):
                    oall_b = state[0]
                    ot = pp.tile([128, 1024], F32, tag="pp",
                                 name=f"ot_{bb}_{eo}_{tb}")[:, 0:512]
                    for ec in range(8):
                        nc.tensor.matmul(
                            ot, oall_b[:, ec, 128 * tb:128 * tb + 128],
                            wo_sb[:, ec, 512 * eo:512 * eo + 512],
                            start=(ec == 0), stop=(ec == 7))
                    ys = outp.tile([128, 512], F32, tag="ys")
                    nc.scalar.copy(ys[:], ot)
                    nc.sync.dma_start(
                        y_d[TPB * bb + 128 * tb:TPB * bb + 128 * tb + 128,
                            512 * eo:512 * eo + 512], ys[:])

                return [first,
                        lambda: one(0, 0), lambda: one(0, 1),
                        lambda: one(1, 0), lambda: one(1, 1)]

            # ============ schedule =========================
            steps = deque()
            for ci in range(2):            # prologue: first two chunks
                for s in proj_chunk_steps(ci):
                    s()
            for ci in range(2, 4):
                steps.extend(proj_chunk_steps(ci))
            for bb in range(B):
                if bb + 1 < B:
                    for ci in range(4 * (bb + 1), 4 * (bb + 1) + 4):
                        steps.extend(proj_chunk_steps(ci))
                if bb >= 1:
                    steps.extend(oproj_steps(bb - 1))
                for c in range(4):
                    attn_chunk(bb, c, steps)
                nc.gpsimd.collective_compute(
                    "AllToAll", mybir.AluOpType.bypass, replica_groups=rg,
                    ins=[a2a_in[bb].opt()], outs=[a2a_out[bb].opt()])
            while steps:                   # leftovers (should be few)
                steps.popleft()()
            for s in oproj_steps(B - 1):   # tail
                s()

    nc.compile()
    return nc


def _host_inputs(x, Wq, Wk, Wv, Wo, token_positions):
    """Per-core in_maps with transposed/tiled layouts."""
    x = np.asarray(x, dtype=np.float32)
    xt_bf = np.ascontiguousarray(
        x.reshape(NT, D).T.reshape(8, 128, NT).transpose(1, 0, 2)
    ).astype(ml_dtypes.bfloat16)

    pos = np.asarray(token_positions).astype(np.float64)
    inv_freq = 1.0 / (THETA ** (np.arange(0, DH, 2, dtype=np.float64) / DH))
    ang = pos[None, :] * inv_freq[:, None]          # [32, T]
    cos_p = np.cos(ang)
    sin_p = np.sin(ang)
    d_idx = (np.arange(128) % 64) // 2
    cosb = cos_p[d_idx, :].astype(np.float32)
    sinb = sin_p[d_idx, :].astype(np.float32)

    rotm = np.zeros((128, 128), dtype=np.float32)
    for i in range(64):
        rotm[2 * i + 1, 2 * i] = -1.0
        rotm[2 * i, 2 * i + 1] = 1.0
    rotm = rotm.astype(ml_dtypes.bfloat16)
    tri = np.tril(np.ones((128, 128), dtype=np.float32)).T  # [tk, tq] tk<=tq
    tri = tri.astype(ml_dtypes.bfloat16)

    def wtiles(W, sl):
        Wt = np.ascontiguousarray(np.asarray(W, np.float32)[sl, :].T)
        return np.ascontiguousarray(
            Wt.reshape(8, 128, Wt.shape[1]).transpose(1, 0, 2)
        ).astype(ml_dtypes.bfloat16)

    WoT = np.ascontiguousarray(np.asarray(Wo, dtype=np.float32).T)
    wo_t = np.ascontiguousarray(
        WoT.reshape(8, 128, D).transpose(1, 0, 2)).astype(ml_dtypes.bfloat16)

    in_maps = []
    for c in range(N_CORES):
        sl = slice(EC * c, EC * (c + 1))
        in_maps.append({
            "xt": xt_bf,
            "wq": wtiles(Wq, sl),
            "wk": wtiles(Wk, sl),
            "wv": wtiles(Wv, sl),
            "wo": wo_t,
            "cosb": cosb,
            "sinb": sinb,
            "rotm": rotm,
            "trimask": tri,
        })
    return in_maps


def kernel(x, Wq, Wk, Wv, Wo, token_positions):
    global last_results
    if "nc" not in _CACHE:
        _CACHE["nc"] = _build_program()
    nc = _CACHE["nc"]
    in_maps = _host_inputs(x, Wq, Wk, Wv, Wo, token_positions)
    res = bass_utils.run_bass_kernel_spmd(nc, in_maps, list(range(N_CORES)))
    last_results = res
    y = np.empty((NT, D), dtype=np.float32)
    for c in range(N_CORES):
        yc = res.results[c]["y"]          # [B*TPB, D]
        for b in range(B):
            y[b * T + TPB * c:b * T + TPB * (c + 1)] = \
                yc[TPB * b:TPB * (b + 1)]
    return y.reshape(B, T, D)


# revision 11
# speedup vs baseline: 1.2591x; 1.2591x over previous
"""Multi-head self-attention (RoPE, causal) on 8 Trainium2 NeuronCores.

Sharding: tensor-parallel over heads. Each core owns 2 of 16 heads:
  - QKV projections column-sharded (each core computes its 128 features)
  - V^T computed directly (x-block stationary) so no PE transposes
  - attention per (batch, head) pair on-core, scores kept transposed
    [tk, tq]; the two heads' K=64 score matmuls run concurrently in the
    PE array (row tiling via base_partition 0/64) into one [128,2,512]
    PSUM tile, so softmax exp is a single ACT op per j-block
  - softmax denominator via a ones-column appended to V^T
  - per-batch AllToAll switches from head- to token-sharding; the output
    projection for batch b overlaps attention of batch b+1
  - projection/output-projection work is sliced into ~8-matmul quanta and
    one quantum is emitted per attention j-iteration, so the PE always
    has ready work while ACT walks the exp stream (keeps HAM at 2.4 GHz)

dtypes: bf16 operands everywhere (FWL stays enabled), fp32 PSUM accum.
"""

from collections import deque

import numpy as np
import ml_dtypes

import concourse.bacc as bacc
import concourse.mybir as mybir
import concourse.tile as tile
from concourse import bass_utils

F32 = mybir.dt.float32
BF16 = mybir.dt.bfloat16

B, T, D = 4, 2048, 1024
H, DH = 16, 64
N_CORES = 8
HPC = H // N_CORES            # heads per core = 2
EC = HPC * DH                 # feature slice per core = 128
NT = B * T                    # 8192 tokens
THETA = 10000.0
NB = T // 128                 # 16 tk blocks per batch
TPB = T // N_CORES            # 256 tokens per (core, batch) after A2A

_CACHE = {}
last_results = None


def _build_program():
    nc = bacc.Bacc("TRN2", debug=False, target_bir_lowering=False,
                   num_devices=N_CORES)

    xt_d = nc.dram_tensor("xt", [128, 8, NT], BF16, kind="ExternalInput")
    wq_d = nc.dram_tensor("wq", [128, 8, EC], BF16, kind="ExternalInput")
    wk_d = nc.dram_tensor("wk", [128, 8, EC], BF16, kind="ExternalInput")
    wv_d = nc.dram_tensor("wv", [128, 8, EC], BF16, kind="ExternalInput")
    wo_d = nc.dram_tensor("wo", [128, 8, D], BF16, kind="ExternalInput")
    cos_d = nc.dram_tensor("cosb", [128, T], F32, kind="ExternalInput")
    sin_d = nc.dram_tensor("sinb", [128, T], F32, kind="ExternalInput")
    rotm_d = nc.dram_tensor("rotm", [128, 128], BF16, kind="ExternalInput")
    tri_d = nc.dram_tensor("trimask", [128, 128], BF16, kind="ExternalInput")
    y_d = nc.dram_tensor("y", [B * TPB, D], F32, kind="ExternalOutput")

    with tile.TileContext(nc) as tc:
        with (
            tc.tile_pool(name="consts", bufs=1) as consts,
            tc.tile_pool(name="big", bufs=1) as big,
            tc.tile_pool(name="xp", bufs=2) as xp,
            tc.tile_pool(name="stage", bufs=2) as stage,
            tc.tile_pool(name="expp", bufs=4) as expp,
            tc.tile_pool(name="outp", bufs=3) as outp,
            tc.tile_pool(name="oall", bufs=2) as oallp,
            tc.tile_pool(name="pp", bufs=2, space="PSUM") as pp,
            tc.tile_pool(name="pgen", bufs=2, space="PSUM") as pgen,
            tc.tile_pool(name="pv", bufs=2, space="PSUM") as pvp,
            tc.tile_pool(name="dram", bufs=1, space="DRAM") as dram,
        ):
            # ---- constants ----
            cos_sb = consts.tile([128, T], F32)
            sin_sb = consts.tile([128, T], F32)
            rotm_sb = consts.tile([128, 128], BF16)
            tri2_sb = consts.tile([128, 2, 128], BF16)
            nc.sync.dma_start(cos_sb[:], cos_d[:, :])
            nc.sync.dma_start(sin_sb[:], sin_d[:, :])
            nc.sync.dma_start(rotm_sb[:], rotm_d[:, :])
            nc.sync.dma_start(tri2_sb[:, 0, :], tri_d[:, :])
            nc.sync.dma_start(tri2_sb[:, 1, :], tri_d[:, :])

            wq_sb = consts.tile([128, 8, EC], BF16)
            wk_sb = consts.tile([128, 8, EC], BF16)
            wv_sb = consts.tile([128, 8, EC], BF16)
            wo_sb = consts.tile([128, 8, D], BF16)
            nc.sync.dma_start(wq_sb[:], wq_d[:, :, :])
            nc.sync.dma_start(wk_sb[:], wk_d[:, :, :])
            nc.sync.dma_start(wv_sb[:], wv_d[:, :, :])
            nc.sync.dma_start(wo_sb[:], wo_d[:, :, :])

            # ---- persistent tensors ----
            qT = big.tile([128, NT], BF16, tag="qT")
            kT = big.tile([128, NT], BF16, tag="kT")
            # V^T per (pair, tk-block): [tk=128, 128] cols 0-63 = v,
            # col 64 = ones (softmax denominator), cols 65-127 unused
            # (full-128 stationary keeps FWL enabled on the PV matmuls).
            vext = big.tile([128, HPC * B, NB, 128], BF16, tag="vext")
            nc.vector.memset(vext[:, :, :, 64], 1.0)

            a2a_in = [dram.tile([N_CORES, 128, TPB], BF16, tag=f"a2ai{b}",
                                name=f"a2ai{b}") for b in range(B)]
            a2a_out = [dram.tile([N_CORES, 128, TPB], BF16, tag=f"a2ao{b}",
                                 name=f"a2ao{b}") for b in range(B)]
            rg = [list(range(N_CORES))]

            # ============ QKV projections + RoPE for one 512-token chunk,
            # sliced into small PE quanta so they weave between attention
            # j-iterations without starving the ACT exp stream.
            def proj_chunk_steps(ci):
                t0 = 512 * ci
                bb = t0 // T
                state = {}

                def load_x():
                    xt = xp.tile([128, 8, 512], BF16, tag="x",
                                 name=f"x_{ci}")
                    nc.sync.dma_start(xt[:], xt_d[:, :, t0:t0 + 512])
                    state["xt"] = xt

                def _proj(w_sb, nm):
                    pt = pgen.tile([128, 512], F32, tag="pg",
                                   name=f"p{nm}_{ci}")
                    for ko in range(8):
                        nc.tensor.matmul(pt, w_sb[:, ko, :],
                                         state["xt"][:, ko, :],
                                         start=(ko == 0), stop=(ko == 7))
                    state["p" + nm] = pt

                def _rope(nm):
                    raw = stage.tile([128, 512], BF16, tag="raw" + nm,
                                     name=f"raw{nm}_{ci}")
                    nc.vector.tensor_copy(raw[:], state["p" + nm])
                    rot = pgen.tile([128, 512], F32, tag="pg",
                                    name=f"r{nm}_{ci}")
                    nc.tensor.matmul(rot, rotm_sb[:], raw[:],
                                     start=True, stop=True)
                    state["raw" + nm] = raw
                    state["rot" + nm] = rot

                def _combine(nm, dest):
                    s0 = t0 % T
                    raw, rot = state["raw" + nm], state["rot" + nm]
                    t1 = stage.tile([128, 512], F32, tag="t1" + nm,
                                    name=f"t1{nm}_{ci}")
                    nc.vector.tensor_tensor(
                        t1[:], raw[:], cos_sb[:, s0:s0 + 512],
                        mybir.AluOpType.mult)
                    t2 = stage.tile([128, 512], F32, tag="t2" + nm,
                                    name=f"t2{nm}_{ci}")
                    nc.vector.tensor_tensor(
                        t2[:], rot[:], sin_sb[:, s0:s0 + 512],
                        mybir.AluOpType.mult)
                    nc.vector.tensor_tensor(
                        dest[:, t0:t0 + 512], t1[:], t2[:],
                        mybir.AluOpType.add)

                def _vt(tb):
                    if tb == 0:
                        state["vt"] = pgen.tile([128, 512], F32, tag="pg",
                                                name=f"vt_{ci}")
                    pvt_ = state["vt"]
                    for ko in range(8):
                        nc.tensor.matmul(
                            pvt_[:, 128 * tb:128 * tb + 128],
                            state["xt"][:, ko, 128 * tb:128 * tb + 128],
                            wv_sb[:, ko, :],
                            start=(ko == 0), stop=(ko == 7))

                def _vcopy(tb):
                    jg = (t0 % T) // 128 + tb
                    src = state["vt"][:, 128 * tb:128 * tb + 128]
                    nc.vector.tensor_copy(
                        vext[:, bb * HPC:bb * HPC + 2, jg, 0:64],
                        src.rearrange("p (h w) -> p h w", h=2))

                return [
                    load_x,
                    lambda: _proj(wq_sb, "q"),
                    lambda: _rope("q"),
                    lambda: (_combine("q", qT), _proj(wk_sb, "k"))[-1],
                    lambda: _rope("k"),
                    lambda: _combine("k", kT),
                    lambda: _vt(0),
                    lambda: _vt(1),
                    lambda: (_vcopy(0), _vt(2))[-1],
                    lambda: (_vcopy(1), _vt(3))[-1],
                    lambda: (_vcopy(2), _vcopy(3))[-1],
                ]

            # ============ attention for one (batch, 512-wide tq chunk)
            def attn_chunk(bb, c, steps):
                boff = bb * T
                q0 = boff + 512 * c
                jmax = 4 * c + 4
                pvt = [pvp.tile([128, 512], F32, tag="pv",
                                name=f"pv_{bb}_{c}_{h}")
                       for h in range(HPC)]
                for j in range(jmax):
                    lo = max(0, 128 * (j - 4 * c))
                    sc = pp.tile([128, 2, 512], F32, tag="pp",
                                 name=f"s_{bb}_{c}_{j}")
                    for h in range(HPC):
                        nc.tensor.matmul(
                            sc[:, h, lo:512],
                            kT[64 * h:64 * h + 64,
                               boff + 128 * j:boff + 128 * j + 128],
                            qT[64 * h:64 * h + 64, q0 + lo:q0 + 512],
                            start=True, stop=True)
                    ex = expp.tile([128, 2, 512], BF16, tag="e",
                                   name=f"e_{bb}_{c}_{j}")
                    nc.scalar.activation(
                        ex[:, :, lo:512], sc[:, :, lo:512],
                        mybir.ActivationFunctionType.Exp, scale=0.125)
                    if j >= 4 * c:  # diagonal block: causal mask
                        nc.vector.tensor_tensor(
                            ex[:, :, lo:lo + 128], ex[:, :, lo:lo + 128],
                            tri2_sb[:], mybir.AluOpType.mult)
                    for h in range(HPC):
                        nc.tensor.matmul(
                            pvt[h][:, lo:512],
                            vext[:, bb * HPC + h, j, :],
                            ex[:, h, lo:512],
                            start=(j == 0), stop=(j == jmax - 1))
                    if steps:
                        steps.popleft()()
                # normalize + stage for A2A
                for h in range(HPC):
                    unn = outp.tile([64, 512], BF16, tag="unn")
                    nc.vector.tensor_copy(unn[:], pvt[h][0:64, :])
                    dn0 = stage.tile([1, 512], F32, tag="dn0",
                                     name=f"dn0_{bb}_{c}_{h}")
                    nc.vector.tensor_copy(dn0[:], pvt[h][64:65, :])
                    rec = stage.tile([1, 512], F32, tag="rec",
                                     name=f"rec_{bb}_{c}_{h}")
                    nc.vector.reciprocal_approx_fast(rec[:], dn0[:])
                    recb = outp.tile([64, 512], F32, tag="recb")
                    nc.gpsimd.partition_broadcast(recb[:], rec[:])
                    ao = outp.tile([64, 512], BF16, tag="ao")
                    nc.vector.tensor_tensor(ao[:], unn[:], recb[:],
                                            mybir.AluOpType.mult)
                    for half in range(2):
                        nc.sync.dma_start(
                            a2a_in[bb][2 * c + half,
                                       64 * h:64 * h + 64, :],
                            ao[:, 256 * half:256 * half + 256])
                for _ in range(2):
                    if steps:
                        steps.popleft()()

            # ============ output projection for one batch (post-A2A)
            def oproj_steps(bb):
                state = [None]

                def first():
                    oall_b = oallp.tile([128, 8, TPB], BF16, tag="oall",
                                        name=f"oall_{bb}")
                    nc.sync.dma_start(
                        oall_b[:], a2a_out[bb][:].rearrange("s p t -> p s t"))
                    state[0] = oall_b

                def one(eo, tb):
                    oall_b = state[0]
                    ot = pgen.tile([128, 512], F32, tag="pg",
                                   name=f"ot_{bb}_{eo}_{tb}")
                    for ec in range(8):
                        nc.tensor.matmul(
                            ot, oall_b[:, ec, 128 * tb:128 * tb + 128],
                            wo_sb[:, ec, 512 * eo:512 * eo + 512],
                            start=(ec == 0), stop=(ec == 7))
                    ys = outp.tile([128, 512], F32, tag="ys")
                    nc.scalar.copy(ys[:], ot)
                    nc.sync.dma_start(
                        y_d[TPB * bb + 128 * tb:TPB * bb + 128 * tb + 128,
                            512 * eo:512 * eo + 512], ys[:])

                return [first,
                        lambda: one(0, 0), lambda: one(0, 1),
                        lambda: one(1, 0), lambda: one(1, 1)]

            # ============ schedule =========================
            steps = deque()
            for ci in range(2):            # prologue: first two chunks
                for s in proj_chunk_steps(ci):
                    s()
            for ci in range(2, 4):
                steps.extend(proj_chunk_steps(ci))
            for bb in range(B):
                if bb + 1 < B:
                    for ci in range(4 * (bb + 1), 4 * (bb + 1) + 4):
                        steps.extend(proj_chunk_steps(ci))
                if bb >= 1:
                    steps.extend(oproj_steps(bb - 1))
                for c in range(4):
                    attn_chunk(bb, c, steps)
                nc.gpsimd.collective_compute(
                    "AllToAll", mybir.AluOpType.bypass, replica_groups=rg,
                    ins=[a2a_in[bb].opt()], outs=[a2a_out[bb].opt()])
            while steps:                   # leftovers (should be few)
                steps.popleft()()
            for s in oproj_steps(B - 1):   # tail
                s()

    nc.compile()
    return nc


def _host_inputs(x, Wq, Wk, Wv, Wo, token_positions):
    """Per-core in_maps with transposed/tiled layouts."""
    x = np.asarray(x, dtype=np.float32)
    xt_bf = np.ascontiguousarray(
        x.reshape(NT, D).T.reshape(8, 128, NT).transpose(1, 0, 2)
    ).astype(ml_dtypes.bfloat16)

    pos = np.asarray(token_positions).astype(np.float64)
    inv_freq = 1.0 / (THETA ** (np.arange(0, DH, 2, dtype=np.float64) / DH))
    ang = pos[None, :] * inv_freq[:, None]          # [32, T]
    cos_p = np.cos(ang)
    sin_p = np.sin(ang)
    d_idx = (np.arange(128) % 64) // 2
    cosb = cos_p[d_idx, :].astype(np.float32)
    sinb = sin_p[d_idx, :].astype(np.float32)

    rotm = np.zeros((128, 128), dtype=np.float32)
    for i in range(64):
        rotm[2 * i + 1, 2 * i] = -1.0
        rotm[2 * i, 2 * i + 1] = 1.0
    rotm = rotm.astype(ml_dtypes.bfloat16)
    tri = np.tril(np.ones((128, 128), dtype=np.float32)).T  # [tk, tq] tk<=tq
    tri = tri.astype(ml_dtypes.bfloat16)

    def wtiles(W, sl):
        Wt = np.ascontiguousarray(np.asarray(W, np.float32)[sl, :].T)
        return np.ascontiguousarray(
            Wt.reshape(8, 128, Wt.shape[1]).transpose(1, 0, 2)
        ).astype(ml_dtypes.bfloat16)

    WoT = np.ascontiguousarray(np.asarray(Wo, dtype=np.float32).T)
    wo_t = np.ascontiguousarray(
        WoT.reshape(8, 128, D).transpose(1, 0, 2)).astype(ml_dtypes.bfloat16)

    in_maps = []
    for c in range(N_CORES):
        sl = slice(EC * c, EC * (c + 1))
        in_maps.append({
            "xt": xt_bf,
            "wq": wtiles(Wq, sl),
            "wk": wtiles(Wk, sl),
            "wv": wtiles(Wv, sl),
            "wo": wo_t,
            "cosb": cosb,
            "sinb": sinb,
            "rotm": rotm,
            "trimask": tri,
        })
    return in_maps


def kernel(x, Wq, Wk, Wv, Wo, token_positions):
    global last_results
    if "nc" not in _CACHE:
        _CACHE["nc"] = _build_program()
    nc = _CACHE["nc"]
    in_maps = _host_inputs(x, Wq, Wk, Wv, Wo, token_positions)
    res = bass_utils.run_bass_kernel_spmd(nc, in_maps, list(range(N_CORES)))
    last_results = res
    y = np.empty((NT, D), dtype=np.float32)
    for c in range(N_CORES):
        yc = res.results[c]["y"]          # [B*TPB, D]
        for b in range(B):
            y[b * T + TPB * c:b * T + TPB * (c + 1)] = \
                yc[TPB * b:TPB * (b + 1)]
    return y.reshape(B, T, D)
